# revision 19
# baseline (speedup 1.0000x reference)
"""Multi-head GAT layer (2 heads, sum-merged) on 8 TRN2 NeuronCores.

Edges are sharded by destination node (12500 dsts/core): segment softmax
and scatter-sum stay core-local (no collectives).

v7: no dma_gather at all. The host expands h_src[src_e], h_dst[dst_e]
and the dst one-hot per edge slot (slot order = dst-sorted edges packed
into supertiles of <=1152 edges spanning <128 dst rows, K=9 chunks).
Per chunk the device computes [z | s] = hsE^T @ [W | a-proj] (130-col
matmul) and accumulates s_dst = hdE^T @ wdst into the same PSUM cols
(chunk stride 170 f32 = 3 chunks per PSUM bank, no bank straddle).
Then wt = exp(leaky_relu(s)) (lrelu on DVE, exp on ACT), wz =
[z*wt | wt], and a one-hot scatter matmul accumulates 128 dst rows +
softmax denominators. The scatter runs one supertile behind the
projection (software pipelining) so the PE never waits on the DVE.
"""

import numpy as np
import ml_dtypes

import concourse.bacc as bacc
import concourse.mybir as mybir
import concourse.tile as tile
from concourse.bass_utils import run_bass_kernel_spmd

F32 = mybir.dt.float32
BF16 = mybir.dt.bfloat16
FP8 = mybir.dt.float8e4

IN = 128          # input feature dim
OUT = 64          # output feature dim per head
H = 2             # heads
K = 9             # chunks (of 128 slots) per supertile
NSLOT = K * 128   # 1152 edge slots per supertile
CST = 170         # psz chunk stride (f32 cols) within a bank
G = 4             # supertiles per DMA group
NCORES = 8

N_SRC = 100000
N_DST = 100000
NDST_C = N_DST // NCORES


def _chunk_col(j):
    return 512 * (j // 3) + CST * (j % 3)


def _pack_core(src_c, dst_local):
    """dst-sorted edges -> supertiles of whole dst segments with
    total<=NSLOT edges and dst span<128."""
    order = np.argsort(dst_local, kind="stable")
    s = np.ascontiguousarray(src_c[order])
    d = np.ascontiguousarray(dst_local[order])
    n = len(d)
    starts = np.flatnonzero(np.r_[True, np.diff(d) != 0])
    ends = np.r_[starts[1:], n]
    segd = d[starts]
    nseg = len(starts)
    out = []
    cur = 0
    while cur < nseg:
        d0 = int(segd[cur])
        elo = int(starts[cur])
        hi = cur
        while (hi + 1 < nseg and int(segd[hi + 1]) - d0 < 128
               and int(ends[hi + 1]) - elo <= NSLOT):
            hi += 1
        ehi = int(ends[hi])
        assert ehi - elo <= NSLOT
        ss, dd = s[elo:ehi], d[elo:ehi]
        ne = ehi - elo
        srcs = np.full(NSLOT, N_SRC, np.int32)       # pad -> zero row
        srcs[:ne] = ss
        dcs = np.full(NSLOT, NDST_C, np.int32)       # pad -> zero row
        dcs[:ne] = dd
        dstrel = np.full(NSLOT, -1, np.int64)
        dstrel[:ne] = dd - d0
        out.append(dict(d0=d0, srcs=srcs, dcs=dcs, dstrel=dstrel,
                        segs=np.unique(dd)))
        cur = hi + 1
    return out


def _pack_all(src_idx, dst_idx):
    per_core = []
    core_of = dst_idx // NDST_C
    for c in range(NCORES):
        m = core_of == c
        per_core.append(_pack_core(src_idx[m], dst_idx[m] - c * NDST_C))
    T = max(len(t) for t in per_core)
    T = (T + G - 1) // G * G
    srcs = np.full((NCORES, T * NSLOT), N_SRC, np.int32)
    dcs = np.full((NCORES, T * NSLOT), NDST_C, np.int32)
    dstrel = np.full((NCORES, T, NSLOT), -1, np.int64)
    remap = np.full((NCORES, NDST_C), -1, np.int32)
    for c in range(NCORES):
        for ti, t in enumerate(per_core[c]):
            srcs[c, ti * NSLOT:(ti + 1) * NSLOT] = t["srcs"]
            dcs[c, ti * NSLOT:(ti + 1) * NSLOT] = t["dcs"]
            dstrel[c, ti] = t["dstrel"]
            d0 = t["d0"]
            remap[c, t["segs"]] = ti * 128 + (t["segs"] - d0)
    # one-hot scatter matrix: ohE[p, (t*K+j)*128 + q] = (dstrel == q)
    ohE = np.zeros((NCORES, T, K, 128, 128), ml_dtypes.float8_e4m3)
    dr3 = dstrel.reshape(NCORES, T, K, 128)              # [c,t,j,p]
    cc, tt, jj, pp = np.nonzero(dr3 >= 0)
    ohE[cc, tt, jj, pp, dr3[cc, tt, jj, pp]] = 1.0
    ohE = np.ascontiguousarray(
        ohE.transpose(0, 3, 1, 2, 4).reshape(NCORES, 128, T * K * 128))
    return srcs, dcs, ohE, remap, T


def _build_program(T):
    nc = bacc.Bacc("TRN2", target_bir_lowering=False, debug=False,
                   num_devices=NCORES)
    hsE = nc.dram_tensor("hsE", [128, T * NSLOT], BF16,
                         kind="ExternalInput").ap()
    hdE = nc.dram_tensor("hdE", [128, T * NSLOT // 2], BF16,
                         kind="ExternalInput").ap()
    wsr = nc.dram_tensor("wsrc", [128, 132], BF16, kind="ExternalInput").ap()
    wds = nc.dram_tensor("wdst", [128, 2], BF16, kind="ExternalInput").ap()
    ohd = nc.dram_tensor("ohE", [128, T * K * 64], BF16,
                         kind="ExternalInput").ap()
    big = nc.dram_tensor("big", [T * 128, OUT], BF16,
                         kind="ExternalOutput").ap()

    AF = mybir.ActivationFunctionType
    ALU = mybir.AluOpType

    with tile.TileContext(nc) as tc:
        with (
            tc.tile_pool(name="const", bufs=1) as cpool,
            tc.tile_pool(name="hs", bufs=6) as hs_pool,
            tc.tile_pool(name="hd", bufs=6) as hd_pool,
            tc.tile_pool(name="wz", bufs=4) as wz_pool,
            tc.tile_pool(name="oh", bufs=6) as oh_pool,
            tc.tile_pool(name="wt", bufs=6) as w_pool,
            tc.tile_pool(name="fl", bufs=4) as f_pool,
            tc.tile_pool(name="ob", bufs=4) as ob_pool,
            tc.tile_pool(name="psz", bufs=2, space="PSUM") as psz_pool,
            tc.tile_pool(name="ps2", bufs=2, space="PSUM") as ps2_pool,
        ):
            wsrc_t = cpool.tile([128, 132], BF16)
            nc.sync.dma_start(out=wsrc_t[:], in_=wsr[:, :])
            wdst_t = cpool.tile([128, 2], BF16)
            nc.sync.dma_start(out=wdst_t[:], in_=wds[:, :])

            pend = []   # supertiles waiting for scatter (depth 2)

            def sc_mm(pv, j):
                nc.tensor.matmul(
                    out=pv["ps2"][:, 0:130],
                    lhsT=pv["ohg"][:, (pv["s"] * K + j) * 64:
                                   (pv["s"] * K + j + 1) * 64].bitcast(FP8),
                    rhs=pv["wzb"][:, j * 130:(j + 1) * 130],
                    start=(j == 0), stop=(j == K - 1))

            def normalize(pv):
                t, ps2 = pv["t"], pv["ps2"]
                rec = f_pool.tile([128, 2], F32, tag="rec")
                nc.vector.reciprocal_approx_fast(
                    out=rec[:], in_=ps2[:, IN:IN + 2])
                o0 = f_pool.tile([128, OUT], F32, tag="o0")
                nc.scalar.mul(o0[:], ps2[:, 0:OUT], rec[:, 0:1])
                ob = ob_pool.tile([128, OUT], BF16)
                nc.vector.scalar_tensor_tensor(
                    out=ob[:], in0=ps2[:, OUT:2 * OUT],
                    scalar=rec[:, 1:2], in1=o0[:],
                    op0=ALU.mult, op1=ALU.add)
                nc.sync.dma_start(
                    out=big[t * 128:(t + 1) * 128, :], in_=ob[:])

            for g in range(T // G):
                hst = hs_pool.tile([128, G * NSLOT], BF16)
                if g == 0:
                    nc.sync.dma_start(
                        out=hst[:, 0:NSLOT], in_=hsE[:, 0:NSLOT])
                    nc.sync.dma_start(
                        out=hst[:, NSLOT:G * NSLOT],
                        in_=hsE[:, NSLOT:G * NSLOT])
                else:
                    nc.sync.dma_start(
                        out=hst[:],
                        in_=hsE[:, g * G * NSLOT:(g + 1) * G * NSLOT])
                hdt = hd_pool.tile([128, G * NSLOT // 2], BF16)
                nc.scalar.dma_start(
                    out=hdt[:], in_=hdE[:, g * G * NSLOT // 2:
                                        (g + 1) * G * NSLOT // 2])
                ohg = oh_pool.tile([128, G * K * 64], BF16)
                nc.scalar.dma_start(
                    out=ohg[:],
                    in_=ohd[:, g * G * K * 64:(g + 1) * G * K * 64])
                for s in range(G):
                    t = g * G + s
                    psz = psz_pool.tile([128, 1536], F32)
                    # scatter for supertile t-2: its wzb/ohg are long ready,
                    # so these matmuls never stall the PE queue. Interleave
                    # with the projection matmuls so consecutive PSUM
                    # accumulates hit different banks.
                    pv = pend[0] if len(pend) == 2 else None
                    for r in range(3):
                        for q in range(3):
                            j = 3 * q + r    # banks 0,1,2
                            co = _chunk_col(j)
                            sl = hst[:, (s * K + j) * 128:
                                     (s * K + j + 1) * 128]
                            nc.tensor.matmul(
                                out=psz[:, co:co + 130], lhsT=sl,
                                rhs=wsrc_t[:, 0:130], start=True, stop=False)
                        for q in range(3):
                            j = 3 * q + r
                            co = _chunk_col(j)
                            nc.tensor.matmul(
                                out=psz[:, co + IN:co + IN + 2],
                                lhsT=hdt[:, (s * K + j) * 64:
                                         (s * K + j + 1) * 64].bitcast(FP8),
                                rhs=wdst_t[:], start=False, stop=True)
                    if pv is not None:
                        ps2t = ps2_pool.tile([128, 512], F32, tag="ps2")
                        pv["ps2"] = ps2t
                        for j in range(K):
                            sc_mm(pv, j)
                    v4 = psz[:].rearrange("p (b c) -> p b c", c=512)[
                        :, :, 0:3 * CST].rearrange(
                        "p b (m c) -> p b m c", c=CST)
                    e1 = w_pool.tile([128, 2 * K], BF16, tag="e1")
                    e13 = e1[:].rearrange("p (b m c) -> p b m c", b=3, c=2)
                    nc.scalar.activation(out=e13, in_=v4[:, :, :, IN:IN + 2],
                                         func=AF.Exp)
                    e2 = w_pool.tile([128, 2 * K], BF16, tag="e2")
                    e23 = e2[:].rearrange("p (b m c) -> p b m c", b=3, c=2)
                    nc.scalar.activation(out=e23, in_=v4[:, :, :, IN:IN + 2],
                                         func=AF.Exp, scale=0.01)
                    wt = w_pool.tile([128, 2 * K], BF16, tag="wt")
                    nc.vector.tensor_tensor(out=wt[:], in0=e1[:], in1=e2[:],
                                            op=ALU.max)
                    wt4 = wt[:].rearrange("p (b m c) -> p b m c", b=3, c=2)
                    wzb = wz_pool.tile([128, K * 130], BF16)
                    wzb4 = wzb[:].rearrange("p (b m c) -> p b m c", b=3,
                                            c=130)
                    for h in range(H):
                        nc.vector.tensor_tensor(
                            out=wzb4[:, :, :, h * OUT:(h + 1) * OUT],
                            in0=v4[:, :, :, h * OUT:(h + 1) * OUT],
                            in1=wt4[:, :, :, h:h + 1].to_broadcast(
                                [128, 3, 3, OUT]),
                            op=ALU.mult)
                    nc.gpsimd.tensor_copy(out=wzb4[:, :, :, IN:IN + 2],
                                          in_=wt4[:])
                    if pv is not None:
                        normalize(pv)
                        pend.pop(0)
                    pend.append(dict(t=t, wzb=wzb, ohg=ohg, s=s, ps2=None))
            for pv in pend:
                ps2t = ps2_pool.tile([128, 512], F32, tag="ps2")
                pv["ps2"] = ps2t
                for j in range(K):
                    sc_mm(pv, j)
                normalize(pv)

    nc.compile()
    return nc


def _prep_inputs(h_src, h_dst, W_src, W_dst, a_w, src_idx, dst_idx):
    wsr = np.zeros((IN, 132), np.float32)
    wsr[:, :H * OUT] = W_src.reshape(H * OUT, IN).T
    a_s, a_d = a_w[:, :OUT], a_w[:, OUT:]
    wsr[:, H * OUT:H * OUT + H] = np.einsum("hod,ho->dh", W_src, a_s)
    wsr = wsr.astype(ml_dtypes.bfloat16)
    wds = np.einsum("hod,ho->dh", W_dst, a_d).astype(ml_dtypes.bfloat16)

    srcs, dcs, ohE, remap, T = _pack_all(
        np.asarray(src_idx), np.asarray(dst_idx))

    hs_pad = np.zeros((N_SRC + 1, IN), ml_dtypes.bfloat16)
    hs_pad[:N_SRC] = h_src.astype(ml_dtypes.bfloat16)

    in_maps = []
    for c in range(NCORES):
        hd_pad = np.zeros((NDST_C + 1, IN), ml_dtypes.float8_e4m3)
        hd_pad[:NDST_C] = h_dst[c * NDST_C:(c + 1) * NDST_C].astype(
            ml_dtypes.bfloat16)
        hsEc = np.ascontiguousarray(hs_pad[srcs[c]].T)
        hdEc = np.ascontiguousarray(hd_pad[dcs[c]].T).view(ml_dtypes.bfloat16)
        in_maps.append({
            "hsE": hsEc,
            "hdE": hdEc,
            "wsrc": wsr,
            "wdst": wds,
            "ohE": np.ascontiguousarray(ohE[c]).view(ml_dtypes.bfloat16),
        })
    return in_maps, remap, T


def _run(inputs, trace=False):
    inputs = {k: np.asarray(v) for k, v in inputs.items()}
    in_maps, remap, T = _prep_inputs(**inputs)
    nc = _build_program(T)
    res = run_bass_kernel_spmd(
        nc, in_maps, core_ids=list(range(NCORES)), trace=trace)
    parts = []
    for c in range(NCORES):
        bigc = np.asarray(res.results[c]["big"]).astype(np.float32)
        outc = np.zeros((NDST_C, OUT), np.float32)
        valid = remap[c] >= 0
        outc[valid] = bigc[remap[c][valid]]
        parts.append(outc)
    return np.concatenate(parts, axis=0), res


def kernel(**inputs):
    out, _ = _run(inputs, trace=False)
    return out
